# revision 35
# baseline (speedup 1.0000x reference)
"""MoE layer (top-2 routing, 8 experts) on 8 Trainium2 NeuronCores.

Hidden-dim sharding for perfect load balance: every core holds an H/8
slice (512 wide) of ALL 8 experts' W1/W2 (16.8 MB bf16, SBUF-resident)
and processes ALL 16384 (token, expert) pairs for its slice — so the
per-core matmul column count is exactly 16384*64 regardless of routing
imbalance, and the SPMD program is bit-identical across cores (only
the weight-slice contents differ per core).

The host computes router logits (it must anyway, to decide the
dispatch), gathers tokens by expert into one x^T stream, and applies
the top-2 combine weights + cross-core reduction during the return
scatter-add.  The device runs only the FFN: per <=512-token sub-tile,
GEMM1 (4 h-chunk chains x 8 d-matmuls) -> exact-GELU(+b1) on the
scalar engine -> GEMM2 (8 d-chunk chains x 4 h-matmuls) -> bf16 cast
on the vector engine -> y^T DMA out.  l1(k+1) is emitted before l2(k)
(one-deep software pipeline) so the PE never waits on the gelu drain.

Measured ~469us/core vs the 437.5us bf16 PE column floor (1,050,112
columns at 2.4 GHz); the rest is framework preamble/epilogue (~14us)
and the DMA-ramp-limited startup.  fp8 was evaluated and rejected:
even quantizing only x to e4m3 gives rel_err 0.021 > the 2e-2 gate.
"""

import sys, os

for _p in ("/root/.axon_site", "/root/.axon_site/_ro/trn_rl_repo",
           "/root/.axon_site/_ro/pypackages", "/opt/trn_rl_repo"):
    if os.path.isdir(_p) and _p not in sys.path:
        sys.path.append(_p)

import numpy as np
import ml_dtypes

BF16 = ml_dtypes.bfloat16

T, DIM, E, K, H = 8192, 1024, 8, 2, 4096
N_CORES = 8
ND = DIM // 128             # 8 d-chunks
HS = H // N_CORES           # 512 hidden dims per core
NHL = HS // 128             # 4 h-chunks per core
LT = 512                    # sub-tile token length

_compiled = {}


def _build(G):
    from concourse import bass, bacc, tile, mybir

    dt = mybir.dt
    R = sum(G)

    # sub-tiles: (expert, global row start, length) — lengths evened out
    # (multiples of 16) so there are no tiny remainder chains at group ends.
    # Group 0 starts with two small tiles (128/256 rows) so the first matmul
    # chain only waits on a ~0.4MB DMA instead of ~0.8MB.
    def split(g, lead):
        rem = g - sum(lead)
        nst = -(-rem // LT)
        base = (rem // nst) // 16 * 16
        nplus = (rem - base * nst) // 16
        return list(lead) + [base + 16] * nplus + [base] * (nst - nplus)

    sts = []
    off = 0
    for e in range(E):
        if e == 0:
            lens = split(G[e], (128, 256))
        elif e == E - 1:
            # small tiles last: the final cast + store drain is short
            lens = split(G[e], (128, 256))[::-1]
        else:
            lens = split(G[e], ())
        s = 0
        for L in lens:
            sts.append((e, off + s, L))
            s += L
        assert s == G[e]
        off += G[e]
    n = len(sts)

    nc = bacc.Bacc("TRN2", target_bir_lowering=False, debug=False,
                   num_devices=N_CORES)

    xT = nc.dram_tensor("xT", [128, ND, R], dt.bfloat16, kind="ExternalInput").ap()
    w1d = nc.dram_tensor("w1d", [E, 128, NHL, ND, 128], dt.bfloat16, kind="ExternalInput").ap()
    w2d = nc.dram_tensor("w2d", [E, 128, ND, NHL, 128], dt.bfloat16, kind="ExternalInput").ap()
    b1d = nc.dram_tensor("b1d", [128, E, NHL], dt.float32, kind="ExternalInput").ap()
    outT = nc.dram_tensor("outT", [128, ND, R], dt.bfloat16, kind="ExternalOutput").ap()

    with tile.TileContext(nc) as tc:
        with tc.tile_pool(name="const", bufs=1) as const, \
             tc.tile_pool(name="res", bufs=1) as res, \
             tc.tile_pool(name="xgp", bufs=5) as xgp, \
             tc.tile_pool(name="hp", bufs=3) as hp, \
             tc.tile_pool(name="op", bufs=2) as op, \
             tc.tile_pool(name="p1", bufs=3, space="PSUM") as p1, \
             tc.tile_pool(name="p2", bufs=5, space="PSUM") as p2:

            b1sb = const.tile([128, E, NHL], dt.float32)
            w1sb = res.tile([128, E, NHL, ND, 128], dt.bfloat16)   # 64KB/part
            w2sb = res.tile([128, E, ND, NHL, 128], dt.bfloat16)   # 64KB/part

            # pre-warm the PE during the initial DMA wait: ~10 dummy matmuls
            # keep the HAM activity monitor busy so the clock gate is already
            # at 2.4 GHz when the first real chain issues (else the first
            # ~3.4us of real work run at 1.2 GHz).
            zsb = const.tile([128, LT], dt.bfloat16)
            zrd = const.tile([128, 4], dt.float32)
            nc.vector.memset(zsb[:], 0.0)
            pwt = p2.tile([128, LT], dt.float32, tag="ps2")
            for _ in range(10):
                nc.tensor.matmul(pwt[:], lhsT=zsb[:, :128], rhs=zsb[:],
                                 start=True, stop=True)
            nc.scalar.copy(zrd[:], pwt[:, :4])

            x_tiles = {}

            def load_x(k):
                _, s, L = sts[k]
                xg = xgp.tile([128, ND, LT], dt.bfloat16, tag="xg")
                nc.sync.dma_start(xg[:, :, :L], xT[:, :, s:s + L])
                x_tiles[k] = xg

            # Single sync HW-DGE ring for everything (the gpsimd SW-DGE ring
            # delivered early weights too late, and scalar-ring stores
            # tangled the tail).  Startup: first W1 chunk + the two small
            # lead x tiles first so the first chain starts within ~3us;
            # the remaining experts' weights are chunked and drip-fed
            # inside the loop so x prefetches never queue behind megabytes
            # of weights (head-of-line block on the FIFO ring).
            nc.sync.dma_start(w1sb[:, 0, 0], w1d[0, :, 0])
            load_x(0)
            load_x(1)
            for hc in range(1, NHL):
                nc.sync.dma_start(w1sb[:, 0, hc], w1d[0, :, hc])
            nc.sync.dma_start(b1sb[:], b1d[:])
            load_x(2)
            nc.sync.dma_start(w2sb[:, 0, :4], w2d[0, :, :4])
            nc.sync.dma_start(w2sb[:, 0, 4:], w2d[0, :, 4:])
            load_x(3)
            nc.sync.dma_start(w1sb[:, 1], w1d[1])
            load_x(4)
            wq = []                       # chunked weight DMAs, consumption order
            for e in range(1, E):
                if e > 1:
                    for hc in range(NHL):
                        wq.append((w1sb[:, e, hc], w1d[e, :, hc]))
                for dc in range(0, ND, 2):
                    wq.append((w2sb[:, e, dc:dc + 2], w2d[e, :, dc:dc + 2]))
            wq.reverse()                  # pop from the end

            h_tiles = {}

            def emit_l1(k):
                e, _, L = sts[k]
                xg = x_tiles[k]
                h = hp.tile([128, NHL, LT], dt.bfloat16, tag="h")
                for hc in range(NHL):
                    ps = p1.tile([128, LT], dt.float32, tag="ps1")
                    for dc in range(ND):
                        nc.tensor.matmul(ps[:, :L], lhsT=w1sb[:, e, hc, dc, :],
                                         rhs=xg[:, dc, :L],
                                         start=(dc == 0), stop=(dc == ND - 1))
                    nc.scalar.activation(h[:, hc, :L], ps[:, :L],
                                         bass.mybir.ActivationFunctionType.Gelu,
                                         bias=b1sb[:, e, hc:hc + 1])
                h_tiles[k] = h

            def emit_l2(k):
                e, s, L = sts[k]
                h = h_tiles.pop(k)
                osb = op.tile([128, ND, LT], dt.bfloat16, tag="osb")
                for dc in range(ND):
                    ps = p2.tile([128, LT], dt.float32, tag="ps2")
                    for hc in range(NHL):
                        nc.tensor.matmul(ps[:, :L], lhsT=w2sb[:, e, dc, hc, :],
                                         rhs=h[:, hc, :L],
                                         start=(hc == 0), stop=(hc == NHL - 1))
                    nc.vector.tensor_scalar_mul(osb[:, dc, :L], ps[:, :L], 1.0)
                    if dc == 3:
                        nc.sync.dma_start(outT[:, :4, s:s + L], osb[:, :4, :L])
                nc.sync.dma_start(outT[:, 4:, s:s + L], osb[:, 4:, :L])

            # two-deep l1 software pipeline: l2(k) runs two sub-tile-times
            # after l1(k), giving the DMA ramp ~20us to deliver W2[e0]
            # before the first l2 chain and doubling the gelu-drain slack.
            emit_l1(0)
            emit_l1(1)
            for k in range(n):
                if k + 5 < n:
                    load_x(k + 5)
                for _ in range(2):
                    if wq:
                        dst, src = wq.pop()
                        nc.sync.dma_start(dst, src)
                if k + 2 < n:
                    emit_l1(k + 2)
                emit_l2(k)

    nc.compile()
    return nc


def _route(x_flat, Wr):
    logits = x_flat @ Wr                                  # [T, E] fp32
    order = np.argsort(-logits, axis=1)
    top2 = order[:, :K]
    gap = (np.take_along_axis(logits, top2[:, 0:1], 1)
           - np.take_along_axis(logits, top2[:, 1:2], 1))[:, 0]
    w1v = 1.0 / (1.0 + np.exp(-gap))                      # softmax over top-2
    w2v = 1.0 - w1v
    idxs, wts = [], []
    for e in range(E):
        sel = (top2[:, 0] == e) | (top2[:, 1] == e)
        idx = np.nonzero(sel)[0]
        idxs.append(idx)
        wts.append(np.where(top2[idx, 0] == e, w1v[idx], w2v[idx]).astype(np.float32))
    combine = np.zeros((x_flat.shape[0], E), np.float32)
    np.put_along_axis(combine, top2[:, 0:1], w1v[:, None].astype(np.float32), 1)
    np.put_along_axis(combine, top2[:, 1:2], w2v[:, None].astype(np.float32), 1)
    return idxs, wts, combine


def kernel(x, Wr, W1, b1, W2, b2, _profile=None):
    global _compiled
    from concourse.bass_utils import run_bass_kernel_spmd

    x_flat = np.ascontiguousarray(np.asarray(x, np.float32)).reshape(T, DIM)
    idxs, wts, combine = _route(x_flat, np.asarray(Wr, np.float32))
    cnts = [len(i) for i in idxs]
    G = tuple(-(-c // 16) * 16 for c in cnts)
    R = sum(G)
    off = np.cumsum([0] + list(G))

    if G not in _compiled:
        _compiled[G] = _build(G)
    nc = _compiled[G]

    W1 = np.asarray(W1, np.float32)
    b1 = np.asarray(b1, np.float32)
    W2 = np.asarray(W2, np.float32)
    b2 = np.asarray(b2, np.float32)

    # gathered token stream, transposed: xT[dp, dc, row]
    Xg = np.zeros((R, DIM), np.float32)
    for e in range(E):
        Xg[off[e]:off[e] + cnts[e]] = x_flat[idxs[e]]
    xT = np.ascontiguousarray(Xg.reshape(R, ND, 128).transpose(2, 1, 0).astype(BF16))

    b1r = b1.reshape(E, N_CORES, NHL, 128)                # [e, core, hc, hp]
    in_maps = []
    for c in range(N_CORES):
        sl = slice(c * HS, (c + 1) * HS)
        w1c = W1[:, :, sl].astype(BF16).reshape(E, ND, 128, NHL, 128)
        w1c = np.ascontiguousarray(w1c.transpose(0, 2, 3, 1, 4))   # [e,dp,hc,dc,hp]
        w2c = W2[:, sl, :].astype(BF16).reshape(E, NHL, 128, ND, 128)
        w2c = np.ascontiguousarray(w2c.transpose(0, 2, 3, 1, 4))   # [e,hp,dc,hc,dp]
        b1c = np.ascontiguousarray(b1r[:, c].transpose(2, 0, 1))   # [hp,e,hc]
        in_maps.append({"xT": xT, "w1d": w1c, "w2d": w2c, "b1d": b1c})

    kwargs = {}
    if _profile:
        kwargs = dict(trace=True, tmpdir=_profile)
    res = run_bass_kernel_spmd(nc, in_maps, core_ids=list(range(N_CORES)), **kwargs)

    acc = np.zeros((128, ND, R), np.float32)
    for c in range(N_CORES):
        acc += np.asarray(res.results[c]["outT"]).astype(np.float32)
    y = acc.transpose(2, 1, 0).reshape(R, DIM)

    full = combine @ b2                                    # [T, D] bias term
    for e in range(E):
        full[idxs[e]] += wts[e][:, None] * y[off[e]:off[e] + cnts[e]]
    full = full.reshape(4, 2048, DIM)
    if _profile:
        return full, res
    return full


# revision 36
# speedup vs baseline: 1.0083x; 1.0083x over previous
"""MoE layer (top-2 routing, 8 experts) on 8 Trainium2 NeuronCores.

Hidden-dim sharding for perfect load balance: every core holds an H/8
slice (512 wide) of ALL 8 experts' W1/W2 (16.8 MB bf16, SBUF-resident)
and processes ALL 16384 (token, expert) pairs for its slice — so the
per-core matmul column count is exactly 16384*64 regardless of routing
imbalance, and the SPMD program is bit-identical across cores (only
the weight-slice contents differ per core).

The host computes router logits (it must anyway, to decide the
dispatch), gathers tokens by expert into one x^T stream, and applies
the top-2 combine weights + cross-core reduction during the return
scatter-add.  The device runs only the FFN: per <=512-token sub-tile,
GEMM1 (4 h-chunk chains x 8 d-matmuls) -> exact-GELU(+b1) on the
scalar engine -> GEMM2 (8 d-chunk chains x 4 h-matmuls) -> bf16 cast
on the vector engine -> y^T DMA out.  l1(k+1) is emitted before l2(k)
(one-deep software pipeline) so the PE never waits on the gelu drain.

Measured ~469us/core vs the 437.5us bf16 PE column floor (1,050,112
columns at 2.4 GHz); the rest is framework preamble/epilogue (~14us)
and the DMA-ramp-limited startup.  fp8 was evaluated and rejected:
even quantizing only x to e4m3 gives rel_err 0.021 > the 2e-2 gate.
"""

import sys, os

for _p in ("/root/.axon_site", "/root/.axon_site/_ro/trn_rl_repo",
           "/root/.axon_site/_ro/pypackages", "/opt/trn_rl_repo"):
    if os.path.isdir(_p) and _p not in sys.path:
        sys.path.append(_p)

import numpy as np
import ml_dtypes

BF16 = ml_dtypes.bfloat16

T, DIM, E, K, H = 8192, 1024, 8, 2, 4096
N_CORES = 8
ND = DIM // 128             # 8 d-chunks
HS = H // N_CORES           # 512 hidden dims per core
NHL = HS // 128             # 4 h-chunks per core
LT = 512                    # sub-tile token length

_compiled = {}


def _build(G):
    from concourse import bass, bacc, tile, mybir

    dt = mybir.dt
    R = sum(G)

    # sub-tiles: (expert, global row start, length) — lengths evened out
    # (multiples of 16) so there are no tiny remainder chains at group ends.
    # Group 0 starts with two small tiles (128/256 rows) so the first matmul
    # chain only waits on a ~0.4MB DMA instead of ~0.8MB.
    def split(g, lead):
        rem = g - sum(lead)
        nst = -(-rem // LT)
        base = (rem // nst) // 16 * 16
        nplus = (rem - base * nst) // 16
        return list(lead) + [base + 16] * nplus + [base] * (nst - nplus)

    sts = []
    off = 0
    for e in range(E):
        if e == 0:
            lens = split(G[e], (128, 256))
        elif e == E - 1:
            # small tiles last: the final cast + store drain is short
            lens = split(G[e], (128, 256))[::-1]
        else:
            lens = split(G[e], ())
        s = 0
        for L in lens:
            sts.append((e, off + s, L))
            s += L
        assert s == G[e]
        off += G[e]
    n = len(sts)

    nc = bacc.Bacc("TRN2", target_bir_lowering=False, debug=False,
                   num_devices=N_CORES)

    xT = nc.dram_tensor("xT", [128, ND, R], dt.bfloat16, kind="ExternalInput").ap()
    w1d = nc.dram_tensor("w1d", [E, 128, NHL, ND, 128], dt.bfloat16, kind="ExternalInput").ap()
    w2d = nc.dram_tensor("w2d", [E, 128, ND, NHL, 128], dt.bfloat16, kind="ExternalInput").ap()
    b1d = nc.dram_tensor("b1d", [128, E, NHL], dt.float32, kind="ExternalInput").ap()
    outT = nc.dram_tensor("outT", [128, ND, R], dt.bfloat16, kind="ExternalOutput").ap()

    with tile.TileContext(nc) as tc:
        with tc.tile_pool(name="const", bufs=1) as const, \
             tc.tile_pool(name="res", bufs=1) as res, \
             tc.tile_pool(name="xgp", bufs=5) as xgp, \
             tc.tile_pool(name="hp", bufs=3) as hp, \
             tc.tile_pool(name="op", bufs=2) as op, \
             tc.tile_pool(name="p1", bufs=3, space="PSUM") as p1, \
             tc.tile_pool(name="p2", bufs=5, space="PSUM") as p2:

            b1sb = const.tile([128, E, NHL], dt.float32)
            w1sb = res.tile([128, E, NHL, ND, 128], dt.bfloat16)   # 64KB/part
            w2sb = res.tile([128, E, ND, NHL, 128], dt.bfloat16)   # 64KB/part

            # pre-warm the PE during the initial DMA wait: ~10 dummy matmuls
            # keep the HAM activity monitor busy so the clock gate is already
            # at 2.4 GHz when the first real chain issues (else the first
            # ~3.4us of real work run at 1.2 GHz).
            zsb = const.tile([128, LT], dt.bfloat16)
            zrd = const.tile([128, 4], dt.float32)
            nc.vector.memset(zsb[:], 0.0)
            pwt = p2.tile([128, LT], dt.float32, tag="ps2")
            for _ in range(10):
                nc.tensor.matmul(pwt[:], lhsT=zsb[:, :128], rhs=zsb[:],
                                 start=True, stop=True)
            nc.scalar.copy(zrd[:], pwt[:, :4])

            x_tiles = {}

            def load_x(k):
                _, s, L = sts[k]
                xg = xgp.tile([128, ND, LT], dt.bfloat16, tag="xg")
                nc.sync.dma_start(xg[:, :, :L], xT[:, :, s:s + L])
                x_tiles[k] = xg

            # Single sync HW-DGE ring for everything (the gpsimd SW-DGE ring
            # delivered early weights too late, and scalar-ring stores
            # tangled the tail).  Startup: first W1 chunk + the two small
            # lead x tiles first so the first chain starts within ~3us;
            # the remaining experts' weights are chunked and drip-fed
            # inside the loop so x prefetches never queue behind megabytes
            # of weights (head-of-line block on the FIFO ring).
            nc.sync.dma_start(w1sb[:, 0, 0], w1d[0, :, 0])
            load_x(0)
            load_x(1)
            for hc in range(1, NHL):
                nc.sync.dma_start(w1sb[:, 0, hc], w1d[0, :, hc])
            nc.sync.dma_start(b1sb[:], b1d[:])
            load_x(2)
            nc.sync.dma_start(w2sb[:, 0, :4], w2d[0, :, :4])
            nc.sync.dma_start(w2sb[:, 0, 4:], w2d[0, :, 4:])
            load_x(3)
            nc.sync.dma_start(w1sb[:, 1], w1d[1])
            load_x(4)
            wq = []                       # chunked weight DMAs, consumption order
            for e in range(1, E):
                if e > 1:
                    for hc in range(NHL):
                        wq.append((w1sb[:, e, hc], w1d[e, :, hc]))
                for dc in range(0, ND, 2):
                    wq.append((w2sb[:, e, dc:dc + 2], w2d[e, :, dc:dc + 2]))
            wq.reverse()                  # pop from the end

            h_tiles = {}

            def emit_l1(k):
                e, _, L = sts[k]
                xg = x_tiles[k]
                h = hp.tile([128, NHL, LT], dt.bfloat16, tag="h")
                for hc in range(NHL):
                    ps = p1.tile([128, LT], dt.float32, tag="ps1")
                    for dc in range(ND):
                        nc.tensor.matmul(ps[:, :L], lhsT=w1sb[:, e, hc, dc, :],
                                         rhs=xg[:, dc, :L],
                                         start=(dc == 0), stop=(dc == ND - 1))
                    nc.scalar.activation(h[:, hc, :L], ps[:, :L],
                                         bass.mybir.ActivationFunctionType.Gelu,
                                         bias=b1sb[:, e, hc:hc + 1])
                h_tiles[k] = h

            def emit_l2(k):
                e, s, L = sts[k]
                h = h_tiles.pop(k)
                osb = op.tile([128, ND, LT], dt.bfloat16, tag="osb")
                for dc in range(ND):
                    ps = p2.tile([128, LT], dt.float32, tag="ps2")
                    for hc in range(NHL):
                        nc.tensor.matmul(ps[:, :L], lhsT=w2sb[:, e, dc, hc, :],
                                         rhs=h[:, hc, :L],
                                         start=(hc == 0), stop=(hc == NHL - 1))
                    nc.vector.tensor_scalar_mul(osb[:, dc, :L], ps[:, :L], 1.0)
                    if dc == 3:
                        nc.sync.dma_start(outT[:, :4, s:s + L], osb[:, :4, :L])
                nc.sync.dma_start(outT[:, 4:, s:s + L], osb[:, 4:, :L])

            emit_l1(0)
            for k in range(n):
                if k + 5 < n:
                    load_x(k + 5)
                for _ in range(2):
                    if wq:
                        dst, src = wq.pop()
                        nc.sync.dma_start(dst, src)
                if k + 1 < n:
                    emit_l1(k + 1)
                emit_l2(k)

    nc.compile()
    return nc


def _route(x_flat, Wr):
    logits = x_flat @ Wr                                  # [T, E] fp32
    order = np.argsort(-logits, axis=1)
    top2 = order[:, :K]
    gap = (np.take_along_axis(logits, top2[:, 0:1], 1)
           - np.take_along_axis(logits, top2[:, 1:2], 1))[:, 0]
    w1v = 1.0 / (1.0 + np.exp(-gap))                      # softmax over top-2
    w2v = 1.0 - w1v
    idxs, wts = [], []
    for e in range(E):
        sel = (top2[:, 0] == e) | (top2[:, 1] == e)
        idx = np.nonzero(sel)[0]
        idxs.append(idx)
        wts.append(np.where(top2[idx, 0] == e, w1v[idx], w2v[idx]).astype(np.float32))
    combine = np.zeros((x_flat.shape[0], E), np.float32)
    np.put_along_axis(combine, top2[:, 0:1], w1v[:, None].astype(np.float32), 1)
    np.put_along_axis(combine, top2[:, 1:2], w2v[:, None].astype(np.float32), 1)
    return idxs, wts, combine


def kernel(x, Wr, W1, b1, W2, b2, _profile=None):
    global _compiled
    from concourse.bass_utils import run_bass_kernel_spmd

    x_flat = np.ascontiguousarray(np.asarray(x, np.float32)).reshape(T, DIM)
    idxs, wts, combine = _route(x_flat, np.asarray(Wr, np.float32))
    cnts = [len(i) for i in idxs]
    G = tuple(-(-c // 16) * 16 for c in cnts)
    R = sum(G)
    off = np.cumsum([0] + list(G))

    if G not in _compiled:
        _compiled[G] = _build(G)
    nc = _compiled[G]

    W1 = np.asarray(W1, np.float32)
    b1 = np.asarray(b1, np.float32)
    W2 = np.asarray(W2, np.float32)
    b2 = np.asarray(b2, np.float32)

    # gathered token stream, transposed: xT[dp, dc, row]
    Xg = np.zeros((R, DIM), np.float32)
    for e in range(E):
        Xg[off[e]:off[e] + cnts[e]] = x_flat[idxs[e]]
    xT = np.ascontiguousarray(Xg.reshape(R, ND, 128).transpose(2, 1, 0).astype(BF16))

    b1r = b1.reshape(E, N_CORES, NHL, 128)                # [e, core, hc, hp]
    in_maps = []
    for c in range(N_CORES):
        sl = slice(c * HS, (c + 1) * HS)
        w1c = W1[:, :, sl].astype(BF16).reshape(E, ND, 128, NHL, 128)
        w1c = np.ascontiguousarray(w1c.transpose(0, 2, 3, 1, 4))   # [e,dp,hc,dc,hp]
        w2c = W2[:, sl, :].astype(BF16).reshape(E, NHL, 128, ND, 128)
        w2c = np.ascontiguousarray(w2c.transpose(0, 2, 3, 1, 4))   # [e,hp,dc,hc,dp]
        b1c = np.ascontiguousarray(b1r[:, c].transpose(2, 0, 1))   # [hp,e,hc]
        in_maps.append({"xT": xT, "w1d": w1c, "w2d": w2c, "b1d": b1c})

    kwargs = {}
    if _profile:
        kwargs = dict(trace=True, tmpdir=_profile)
    res = run_bass_kernel_spmd(nc, in_maps, core_ids=list(range(N_CORES)), **kwargs)

    acc = np.zeros((128, ND, R), np.float32)
    for c in range(N_CORES):
        acc += np.asarray(res.results[c]["outT"]).astype(np.float32)
    y = acc.transpose(2, 1, 0).reshape(R, DIM)

    full = combine @ b2                                    # [T, D] bias term
    for e in range(E):
        full[idxs[e]] += wts[e][:, None] * y[off[e]:off[e] + cnts[e]]
    full = full.reshape(4, 2048, DIM)
    if _profile:
        return full, res
    return full
